# revision 55
# baseline (speedup 1.0000x reference)
"""Trainium2 Bass kernel for nn_Attention_80092550136278.

Gated attention with pair bias:
  q = (q_data @ Wq) * d^-0.5 ; k = k_data @ Wk ; v = v_data @ Wv   (per head)
  w = softmax(q k^T + pair_bias) ; ctx = w @ v
  out = (ctx * sigmoid(q_data @ Wg.T + gating_b)) @ Wo.T + o_bias

Sharding: 2D - 4 q-shards x 2 head-groups over 8 cores. Core c handles head
group g = c % 2 (4 heads) and q rows s = c // 2 (512 rows). The two
head-group partials of each q slice are summed on the host during the gather
(head-parallel "all-reduce"); o_bias is zeroed on g=1 cores.

v4 structure (from the v3 trace: 25us prologue + 48us stalling main loop +
19.5us serial tail at 92.4us total; ACT-exp is the fundamental floor at
~34us/core):

 - Logits computed transposed [k, q]: pl[128k, 2h*512q] = bias + k_projT^T
   @ q_projT per 128-row k chunk (KC=16 chunks x 2 head-halves = 32 tiles).
 - Bias is fed three ways, round-robin, to balance engines below the ACT
   exp rate (~1.08us/tile): PE identity-matmul inject (12), DVE tensor_add
   (12), GPSIMD tensor_add (8).
 - ctx accumulators are column-packed with tile_position=(0, 64*j): heads
   2*half+j of one half share ONE psum bank [33+33 rows at partition 0/64],
   so pctx = 2 banks total (v3: 4), freeing pl_pool to 3 bufs (6 banks) for
   a deeper logits pipeline.
 - Single ACT table load: the gate sigmoid is computed via the Exp table
   (exp(-x) with scale=-1, then 1/(1+e) on DVE), not the Sigmoid table.
 - Prologue: host-packed DMA layouts (one start for all of Wq/Wk/Wv/WgT;
   contiguous per-chunk kT/vT), projections share the pl psum pool, and the
   attention pipeline starts right after chunk-0 projections.
 - Tail: the 4 softmax denominators (rows 32/96 of the pctx banks) are
   broadcast by 4 packed K=1 matmuls into one [128,512] psum bank, one
   full-width reciprocal, grs = gate * recip, comb[128hd, 512q] = ctx * grs
   (bf16), then the output projection is 4 head-packed K=128 matmuls
   (stationary comb [128, 128q], moving WoT [128, 256]) + K=1 o_bias rank-1
   accumulate, ACT psum->sbuf copy, 2 DMA starts per 128-row block.

All heavy tensors bf16 (halved HBM traffic; bias is 8.4 MB/core); PSUM
accumulation fp32.
"""

import numpy as np

H, D, NQT, NK, C = 8, 32, 2048, 2048, 256
NQ = 512               # q rows per core (4 q-shards)
HG = 4                 # heads per core (2 head groups)
KC = NK // 128         # 16 k-chunks
NHALF = 2 * KC         # 32 half-iterations (2 head-pairs per chunk)
SCALE = D ** -0.5

_CACHE = {}
CTX_PACKED = True   # heads col-packed into 2 psum banks via tile_position
DEBUG = False       # add intermediate dumps (set before first kernel() call)

# Per-half bias path. The cost model's PE p-state ramp (full 2.4 GHz only
# after >3us of CONTINUOUS busy; any idle gap resets to 1.2/0.65 GHz) makes
# an under-committed PE catastrophic: each matmul then costs 2-3x and sits on
# the exp critical path. All-inject keeps the PE saturated slightly above the
# ACT exp rate (1138ns vs ~1060ns per half at full speed), so it stays ramped
# and the loop is PE-bound just above the ACT floor. DVE/GPSIMD stay free.
PATTERN = ["inj"] * 16


def _build_nc():
    import concourse.bass as bass
    import concourse.bacc as bacc
    import concourse.tile as tile
    import concourse.mybir as mybir

    F32 = mybir.dt.float32
    F32R = mybir.dt.float32r
    BF16 = mybir.dt.bfloat16
    AF = mybir.ActivationFunctionType

    nc = bacc.Bacc("TRN2", debug=False)

    # ---- DRAM I/O (per core: head group g, q slice s) ----
    d_qT = nc.dram_tensor("qT", [128, 1024], BF16, kind="ExternalInput")
    d_wall = nc.dram_tensor("wall", [128, 1024], BF16, kind="ExternalInput")
    d_kTc = nc.dram_tensor("kTc", [128, 4096], BF16, kind="ExternalInput")
    d_vTc = nc.dram_tensor("vTc", [128, 4096], BF16, kind="ExternalInput")
    d_biasT = nc.dram_tensor("biasT", [KC, 128, HG * NQ], BF16, kind="ExternalInput")
    d_woT = nc.dram_tensor("woT", [128, C], BF16, kind="ExternalInput")
    d_ones = nc.dram_tensor("ones", [128, 128], F32R, kind="ExternalInput")
    d_ngb = nc.dram_tensor("ngb", [128, 1], F32, kind="ExternalInput")
    d_id = nc.dram_tensor("ident", [128, 128], BF16, kind="ExternalInput")
    d_out = nc.dram_tensor("out", [NQ, C], F32, kind="ExternalOutput")
    if DEBUG:
        d_dbg_qp = nc.dram_tensor("dbg_qp", [128, NQ], BF16, kind="ExternalOutput")
        d_dbg_kp = nc.dram_tensor("dbg_kp", [128, 4 * 512], BF16, kind="ExternalOutput")
        d_dbg_va = nc.dram_tensor("dbg_va", [128, 528], BF16, kind="ExternalOutput")
        d_dbg_wT = nc.dram_tensor("dbg_wT", [3, 128, 1024], BF16, kind="ExternalOutput")
        d_dbg_g = nc.dram_tensor("dbg_g", [128, NQ], F32, kind="ExternalOutput")
        d_dbg_den = nc.dram_tensor("dbg_den", [128, 1024], F32, kind="ExternalOutput")
        d_dbg_rsr = nc.dram_tensor("dbg_rsr", [128, 512], F32, kind="ExternalOutput")
        d_dbg_comb = nc.dram_tensor("dbg_comb", [128, 512], BF16, kind="ExternalOutput")

    with tile.TileContext(nc) as tc:
        with tc.tile_pool(name="persist", bufs=1) as pers:

            # ---------------- persistent SBUF ----------------
            qT_sb = pers.tile([128, 1024], BF16, name="qT_sb")
            wall_sb = pers.tile([128, 1024], BF16, name="wall_sb")
            kTc_sb = pers.tile([128, 4096], BF16, name="kTc_sb")
            vTc_sb = pers.tile([128, 4096], BF16, name="vTc_sb")
            q_projT = pers.tile([128, NQ], BF16, name="q_projT")
            k_projT = [pers.tile([128, 512], BF16, name=f"k_projT{c}")
                       for c in range(4)]
            v_aug = [pers.tile([128, 4 * HG * 33], BF16, name=f"v_aug{c}")
                     for c in range(4)]
            e_sb = pers.tile([128, NQ], BF16, name="e_sb")       # exp(-(gate pre))
            t_sb = pers.tile([128, NQ], F32, name="t_sb")        # 1 + e
            g_sb = pers.tile([128, NQ], F32, name="g_sb")        # sigmoid gate
            woT_sb = pers.tile([128, C], BF16, name="woT_sb")
            ones_sb = pers.tile([128, 128], F32R, name="ones_sb")
            ngb_sb = pers.tile([128, 1], F32, name="ngb_sb")     # -gating_b
            id_sb = pers.tile([128, 128], BF16, name="id_sb")
            denTP = pers.tile([128, 1024], F32R, name="denTP")
            rsr = pers.tile([128, 512], F32, name="rsr")
            grs = pers.tile([128, 512], F32, name="grs")
            comb = pers.tile([128, 512], BF16, name="comb")
            out_sb = [pers.tile([128, C], F32, name=f"out_sb{i}") for i in range(4)]

            ones_r = ones_sb[:]
            ngb_ap = ngb_sb[:]

            # pools
            bias_pool = tc.alloc_tile_pool(name="bias_sb", bufs=8)
            wT_pool = tc.alloc_tile_pool(name="wT_sb", bufs=6)
            s_pool = tc.alloc_tile_pool(name="s_sb", bufs=4)
            pl_pool = tc.alloc_tile_pool(name="pl", bufs=(3 if CTX_PACKED else 2),
                                         space="PSUM")
            pctx_pool = tc.alloc_tile_pool(name="pctx", bufs=1, space="PSUM")
            pctx = [pctx_pool.tile([128, NQ if CTX_PACKED else 2 * NQ], F32,
                                   name=f"pctx{i}") for i in range(2)]

            def ctx_ap2(h, r0, r1):
                # head h of pair h//2 lives at partition 64*(h%2) of its bank
                return pctx[h // 2][64 * (h % 2) + r0:64 * (h % 2) + r1, :]

            bias_tiles = {}

            # half-major iteration order: i = 0..31 -> (half = i//16 head
            # pair, kc = i%16). Heads 0/1 finish accumulating at i=15, so
            # their den/recip/gate/comb chain runs DURING the second
            # half-loop on otherwise-idle DVE/ACT slots.
            def ihk(i):
                return i % 16, i // 16

            def emit_dma(i):
                kc, half = ihk(i)
                bt = bias_pool.tile([128, 2 * NQ], BF16, tag="bias", name="bias_t")
                nc.sync.dma_start(
                    bt[:], d_biasT.ap()[kc, :, half * 2 * NQ:(half + 1) * 2 * NQ])
                bias_tiles[i] = bt

            # ---------------- prologue DMAs ----------------
            # sync (SP) queue in submission-priority order: DMA engines drain
            # descriptors roughly FIFO, so arrival tracks submission. q-side
            # first (gates the first matmul), then k/v chunks interleaved
            # with the first bias tiles; gpsimd queue: tail-only constants.
            nc.sync.dma_start(qT_sb[:], d_qT.ap()[:])
            nc.sync.dma_start(wall_sb[:], d_wall.ap()[:])
            nc.sync.dma_start(ngb_sb[:], d_ngb.ap()[:])
            nc.gpsimd.dma_start(id_sb[:], d_id.ap()[:])
            nc.gpsimd.dma_start(ones_sb[:], d_ones.ap()[:])
            nc.gpsimd.dma_start(woT_sb[:], d_woT.ap()[:])
            nc.sync.dma_start(kTc_sb[:], d_kTc.ap()[:])
            nc.sync.dma_start(vTc_sb[:], d_vTc.ap()[:])
            for i in range(8):
                emit_dma(i)

            # weight slice views within wall_sb: [wq0 wq1 wk0 wk1 wv0 wv1 wg0 wg1]
            def wsl(idx):
                return wall_sb[:, idx * 128:(idx + 1) * 128]

            # v_aug ones columns (position 32 of each 33-wide block)
            v_aug4w = [v_aug[c].rearrange("p (n h e) -> p n h e", n=4, h=HG)
                       for c in range(4)]
            for c in range(4):
                nc.vector.tensor_copy(
                    v_aug4w[c][:, :, :, D:D + 1],
                    ones_r[:, 0:4 * HG].bitcast(F32).rearrange(
                        "p (n h) -> p n h", n=4).unsqueeze(-1))

            # ---------------- projections ----------------
            # q_proj + gate share one pl tile; per-chunk k/v share one pl tile
            def emit_qg():
                pqg = pl_pool.tile([128, 2 * NQ], F32, tag="pl", name="pqg")
                pq, pg = pqg[:, 0:NQ], pqg[:, NQ:2 * NQ]
                for kk in range(2):
                    nc.tensor.matmul(pq, wsl(kk), qT_sb[:, kk * 512:(kk + 1) * 512],
                                     start=(kk == 0), stop=(kk == 1))
                for kk in range(2):
                    nc.tensor.matmul(pg, wsl(6 + kk), qT_sb[:, kk * 512:(kk + 1) * 512],
                                     start=(kk == 0), stop=(kk == 1))
                nc.vector.tensor_copy(q_projT[:], pq)
                # gate: e = exp(-(pre + gb)) on the Exp table; finish on DVE
                nc.scalar.activation(e_sb[:], pg, AF.Exp, bias=ngb_ap, scale=-1.0)

            def emit_proj(c):
                pkv = pl_pool.tile([128, 2 * NQ], F32, tag="pl", name="pkv")
                pk, pv = pkv[:, 0:512], pkv[:, 512:1024]
                for kk in range(2):
                    nc.tensor.matmul(pk, wsl(2 + kk),
                                     kTc_sb[:, 1024 * c + kk * 512:
                                            1024 * c + (kk + 1) * 512],
                                     start=(kk == 0), stop=(kk == 1))
                for j in range(4):
                    for kk in range(2):
                        nc.tensor.matmul(pv[:, j * 128:(j + 1) * 128],
                                         vTc_sb[:, 1024 * c + kk * 512 + j * 128:
                                                1024 * c + kk * 512 + j * 128 + 128],
                                         wsl(4 + kk), start=(kk == 0), stop=(kk == 1))
                nc.vector.tensor_copy(k_projT[c][:], pk)
                nc.vector.tensor_copy(
                    v_aug4w[c][:, :, :, 0:D],
                    pv.rearrange("p (n h d) -> p n h d", n=4, h=HG))

            # ---------------- attention pipeline ----------------
            front = {}

            def emit_front(i):
                kc, half = ihk(i)
                path = PATTERN[i % 16]
                bt = bias_tiles[i]
                pl = pl_pool.tile([128, 2 * NQ], F32, tag="pl", name="pl")
                if path == "inj":
                    for j in range(2):
                        nc.tensor.matmul(pl[:, j * NQ:(j + 1) * NQ], id_sb[:],
                                         bt[:, j * NQ:(j + 1) * NQ],
                                         start=True, stop=False)
                for j in range(2):
                    h = half * 2 + j
                    po = h * 32
                    nc.tensor.matmul(pl[:, j * NQ:(j + 1) * NQ],
                                     k_projT[kc // 4][po:po + 32,
                                                     (kc % 4) * 128:(kc % 4) * 128 + 128],
                                     q_projT[po:po + 32, :],
                                     start=(path != "inj"), stop=True,
                                     tile_position=(po, 0))
                front[i] = (bt, pl)

            def emit_back(i):
                kc, half = ihk(i)
                path = PATTERN[i % 16]
                bt, pl = front.pop(i)
                wT = wT_pool.tile([128, 2 * NQ], BF16, tag="wT", name="wT")
                if path == "inj":
                    nc.scalar.activation(wT[:], pl[:], AF.Exp)
                elif path == "dve":
                    s_t = s_pool.tile([128, 2 * NQ], BF16, tag="s", name="s_t")
                    nc.vector.tensor_add(s_t[:], pl[:], bt[:])
                    nc.scalar.activation(wT[:], s_t[:], AF.Exp)
                else:  # pool: bt holds exp(bias); wT = exp(logits) * expb
                    s_t = s_pool.tile([128, 2 * NQ], BF16, tag="s", name="s_t")
                    nc.scalar.activation(s_t[:], pl[:], AF.Exp)
                    nc.gpsimd.tensor_mul(wT[:], s_t[:], bt[:])
                if DEBUG and i < 3:
                    nc.sync.dma_start(d_dbg_wT.ap()[i], wT[:])
                for j in range(2):
                    h = half * 2 + j
                    co = ((kc % 4) * HG + h) * 33
                    if CTX_PACKED:
                        nc.tensor.matmul(pctx[half][64 * j:64 * j + 33, :],
                                         v_aug[kc // 4][:, co:co + 33],
                                         wT[:, j * NQ:(j + 1) * NQ],
                                         start=(kc == 0), stop=(kc == KC - 1),
                                         tile_position=(0, 64 * j))
                    else:
                        nc.tensor.matmul(pctx[h // 2][0:33,
                                                      (h % 2) * NQ:(h % 2) * NQ + NQ],
                                         v_aug[kc // 4][:, co:co + 33],
                                         wT[:, j * NQ:(j + 1) * NQ],
                                         start=(kc == 0), stop=(kc == KC - 1))

            # q/gate + chunk-0 projections, then the pipeline starts; chunk
            # 1-3 projections are interleaved into the loop (their kTc DMAs
            # land just in time, and all-inj keeps the PE dense regardless)
            emit_qg()
            emit_proj(0)
            nc.vector.tensor_scalar_add(t_sb[:], e_sb[:], 1.0)
            nc.vector.reciprocal_approx_fast(out=g_sb[:], in_=t_sb[:])

            ctxA = pers.tile([128, 512], F32, name="ctxA")

            emit_front(0)
            emit_front(1)
            emit_proj(1)
            emit_proj(2)
            emit_proj(3)
            for i in range(NHALF):
                if i + 8 < NHALF:
                    emit_dma(i + 8)
                emit_back(i)
                if i == 16:
                    # heads 0/1 are final: stage dens at their OWN partitions
                    # (on DVE: ACT is the exp stream, don't perturb it)
                    nc.vector.tensor_copy(denTP[32:33, 0:512], ctx_ap2(0, 32, 33))
                    nc.vector.tensor_copy(denTP[96:97, 0:512], ctx_ap2(1, 32, 33))
                elif i == 17:
                    nc.vector.tensor_copy(ctxA[0:32, :], ctx_ap2(0, 0, 32))
                    nc.vector.tensor_copy(ctxA[32:64, :], ctx_ap2(1, 0, 32))
                elif i == 18:
                    # pctx[0] bank is now free: broadcast den h0 into rows
                    # 0:32 (K=1 rank-1 at tile row 32, baseline-proven combo)
                    nc.tensor.matmul(pctx[0][0:32, :], ones_r[32:33, 0:32],
                                     denTP[32:33, 0:512], start=True, stop=True,
                                     tile_position=(32, 0))
                elif i == 19:
                    nc.vector.tensor_copy(rsr[0:32, :], pctx[0][0:32, :])
                elif i == 20:
                    nc.tensor.matmul(pctx[0][0:32, :], ones_r[96:97, 0:32],
                                     denTP[96:97, 0:512], start=True, stop=True,
                                     tile_position=(96, 0))
                elif i == 21:
                    nc.vector.tensor_copy(rsr[32:64, :], pctx[0][0:32, :])
                elif i == 22:
                    nc.vector.reciprocal_approx_fast(out=rsr[0:64, :],
                                                     in_=rsr[0:64, :])
                    nc.vector.tensor_mul(grs[0:64, :], rsr[0:64, :],
                                         g_sb[0:64, :])
                elif i == 23:
                    nc.vector.tensor_mul(comb[0:32, :], ctxA[0:32, :],
                                         grs[0:32, :])
                    nc.vector.tensor_mul(comb[32:64, :], ctxA[32:64, :],
                                         grs[32:64, :])
                if i + 2 < NHALF:
                    emit_front(i + 2)

            # ---------------- tail (head pair 1 only) ----------------
            # heads 0/1 were fully reduced inside the loop (hooks i=16..23);
            # here: den h2/h3 -> serial rank-1 broadcasts through the freed
            # pctx[0] rows 0:32 -> rsr rows 64:128 -> one in-place recip
            # (rows 0:63 stale but unread) -> grs -> comb rows 64:128.
            nc.scalar.copy(denTP[32:33, 512:1024], ctx_ap2(2, 32, 33))
            nc.vector.tensor_copy(denTP[96:97, 512:1024], ctx_ap2(3, 32, 33))
            prsb = pl_pool.tile([128, 2 * NQ], F32, tag="pl", name="prsb")
            nc.tensor.matmul(prsb[0:32, 0:512], ones_r[32:33, 0:32],
                             denTP[32:33, 512:1024], start=True, stop=True,
                             tile_position=(32, 0))
            nc.tensor.matmul(prsb[0:32, 512:1024], ones_r[96:97, 0:32],
                             denTP[96:97, 512:1024], start=True, stop=True,
                             tile_position=(96, 0))
            nc.vector.tensor_copy(rsr[64:96, :], prsb[0:32, 0:512])
            nc.vector.tensor_copy(rsr[96:128, :], prsb[0:32, 512:1024])
            nc.vector.reciprocal_approx_fast(out=rsr[:], in_=rsr[:])
            nc.vector.tensor_mul(grs[64:128, :], rsr[64:128, :], g_sb[64:128, :])
            nc.vector.tensor_mul(comb[64:96, :], ctx_ap2(2, 0, 32),
                                 grs[64:96, :])
            nc.vector.tensor_mul(comb[96:128, :], ctx_ap2(3, 0, 32),
                                 grs[96:128, :])

            if DEBUG:
                nc.sync.dma_start(d_dbg_qp.ap()[:], q_projT[:])
                for c in range(4):
                    nc.sync.dma_start(d_dbg_kp.ap()[:, c * 512:(c + 1) * 512],
                                      k_projT[c][:])
                nc.sync.dma_start(d_dbg_va.ap()[:], v_aug[0][:])
                nc.sync.dma_start(d_dbg_g.ap()[:], g_sb[:])
                nc.sync.dma_start(d_dbg_den.ap()[:], denTP[:].bitcast(F32))
                nc.sync.dma_start(d_dbg_rsr.ap()[:], rsr[:])
                nc.sync.dma_start(d_dbg_comb.ap()[:], comb[:])

            # out[512, 256] = comb^T @ woT, head-packed K=128 (o_bias is
            # added on the host during the gather)
            for qm in range(4):
                pout = pl_pool.tile([128, 2 * NQ], F32, tag="pl", name="pout")
                nc.tensor.matmul(pout[:, 0:C],
                                 comb[:, qm * 128:qm * 128 + 128], woT_sb[:],
                                 start=True, stop=True)
                if qm % 2 == 0:
                    nc.scalar.copy(out_sb[qm][:], pout[:, 0:C])
                else:
                    nc.vector.tensor_copy(out_sb[qm][:], pout[:, 0:C])
                q0 = qm * 128
                nc.scalar.dma_start(d_out.ap()[q0:q0 + 64, :], out_sb[qm][0:64, :])
                nc.gpsimd.dma_start(d_out.ap()[q0 + 64:q0 + 128, :],
                                    out_sb[qm][64:128, :])

            pctx_pool.release()
            pl_pool.release()
            s_pool.release()
            wT_pool.release()
            bias_pool.release()

    nc.compile()
    return nc


def _prep_in_maps(inputs):
    import ml_dtypes
    BF = ml_dtypes.bfloat16
    q_data = np.asarray(inputs["q_data"], dtype=np.float32)
    k_data = np.asarray(inputs["k_data"], dtype=np.float32)
    v_data = np.asarray(inputs["v_data"], dtype=np.float32)
    pair_bias = np.asarray(inputs["pair_bias"], dtype=np.float32)
    Wq = np.asarray(inputs["Wq"], dtype=np.float32)
    Wk = np.asarray(inputs["Wk"], dtype=np.float32)
    Wv = np.asarray(inputs["Wv"], dtype=np.float32)
    Wg = np.asarray(inputs["Wg"], dtype=np.float32)
    Wo = np.asarray(inputs["Wo"], dtype=np.float32)
    gating_b = np.asarray(inputs["gating_b"], dtype=np.float32)
    o_bias = np.asarray(inputs["o_bias"], dtype=np.float32)

    wq_s = (Wq * np.float32(SCALE)).astype(BF)
    wk_b = Wk.astype(BF)
    wv_b = Wv.astype(BF)
    wgT_b = np.ascontiguousarray(Wg.T).astype(BF)
    woT = np.ascontiguousarray(Wo.T).astype(BF)          # [H*D, CO] hd-major
    # [chunk, 128, kk*512+kq] layouts for k/v (contiguous per chunk)
    kTc = np.ascontiguousarray(
        k_data.reshape(4, 512, 2, 128).transpose(3, 0, 2, 1)).reshape(
        128, 4096).astype(BF)
    vTc = np.ascontiguousarray(
        v_data.reshape(4, 512, 2, 128).transpose(3, 0, 2, 1)).reshape(
        128, 4096).astype(BF)
    # [k, h, q] bias, bf16, shared across cores (cores slice heads + q rows)
    pbT = np.ascontiguousarray(pair_bias.transpose(2, 0, 1)).astype(BF)
    ident = np.eye(128, dtype=np.float32).astype(BF)

    in_maps = []
    for c in range(8):
        g, s = c % 2, c // 2
        hsl = slice(g * HG, (g + 1) * HG)
        csl = slice(g * 128, (g + 1) * 128)
        qs = slice(s * NQ, (s + 1) * NQ)
        qT = q_data[qs, :].T                               # [256, 512]
        qTr = np.ascontiguousarray(
            qT.reshape(2, 128, NQ).transpose(1, 0, 2)).reshape(128, 1024)
        wall = np.concatenate(
            [wq_s[0:128, csl], wq_s[128:256, csl],
             wk_b[0:128, csl], wk_b[128:256, csl],
             wv_b[0:128, csl], wv_b[128:256, csl],
             wgT_b[0:128, csl], wgT_b[128:256, csl]], axis=1)
        biasT = np.ascontiguousarray(pbT[:, hsl, qs]).reshape(
            KC, 128, 2, 2 * NQ).astype(np.float32)
        # "pool"-path halves carry exp(bias) (consumed as a multiplicative
        # factor after exp(logits); GPSIMD cannot read PSUM)
        for i in range(NHALF):
            if PATTERN[i % 16] == "pool":
                kc, half = i // 2, i % 2
                biasT[kc, :, half] = np.exp(biasT[kc, :, half])
        biasT = biasT.reshape(KC, 128, HG * NQ).astype(BF)
        ngb = -np.ascontiguousarray(gating_b[hsl]).reshape(128, 1)
        in_maps.append(dict(
            qT=qTr.astype(BF), wall=np.ascontiguousarray(wall),
            kTc=kTc, vTc=vTc, biasT=biasT,
            woT=np.ascontiguousarray(woT[csl.start:csl.stop, :]),
            ones=np.ones((128, 128), dtype=np.float32),
            ngb=ngb, ident=ident,
        ))
    return in_maps


def _get_nc():
    if "nc" not in _CACHE:
        _CACHE["nc"] = _build_nc()
    return _CACHE["nc"]


def _run(inputs, trace=False, trace_cores=None):
    from concourse import bass_utils
    nc = _get_nc()
    in_maps = _prep_in_maps(inputs)
    kwargs = {}
    if trace:
        kwargs = dict(trace=True, trace_cores=trace_cores or [0])
    res = bass_utils.run_bass_kernel_spmd(nc, in_maps, core_ids=list(range(8)), **kwargs)
    # gather: sum the two head-group partials of each q slice
    out = np.concatenate(
        [res.results[2 * s]["out"] + res.results[2 * s + 1]["out"] for s in range(4)],
        axis=0)
    out += np.asarray(inputs["o_bias"], dtype=np.float32)[None, :]
    return out, res


def kernel(**inputs) -> np.ndarray:
    out, _ = _run(inputs)
    return out


# revision 56
# speedup vs baseline: 1.0768x; 1.0768x over previous
"""Trainium2 Bass kernel for nn_Attention_80092550136278.

Gated attention with pair bias:
  q = (q_data @ Wq) * d^-0.5 ; k = k_data @ Wk ; v = v_data @ Wv   (per head)
  w = softmax(q k^T + pair_bias) ; ctx = w @ v
  out = (ctx * sigmoid(q_data @ Wg.T + gating_b)) @ Wo.T + o_bias

Sharding: 2D - 4 q-shards x 2 head-groups over 8 cores. Core c handles head
group g = c % 2 (4 heads) and q rows s = c // 2 (512 rows). The two
head-group partials of each q slice are summed on the host during the gather
(head-parallel "all-reduce"); o_bias is zeroed on g=1 cores.

v4 structure (from the v3 trace: 25us prologue + 48us stalling main loop +
19.5us serial tail at 92.4us total; ACT-exp is the fundamental floor at
~34us/core):

 - Logits computed transposed [k, q]: pl[128k, 2h*512q] = bias + k_projT^T
   @ q_projT per 128-row k chunk (KC=16 chunks x 2 head-halves = 32 tiles).
 - Bias is fed three ways, round-robin, to balance engines below the ACT
   exp rate (~1.08us/tile): PE identity-matmul inject (12), DVE tensor_add
   (12), GPSIMD tensor_add (8).
 - ctx accumulators are column-packed with tile_position=(0, 64*j): heads
   2*half+j of one half share ONE psum bank [33+33 rows at partition 0/64],
   so pctx = 2 banks total (v3: 4), freeing pl_pool to 3 bufs (6 banks) for
   a deeper logits pipeline.
 - Single ACT table load: the gate sigmoid is computed via the Exp table
   (exp(-x) with scale=-1, then 1/(1+e) on DVE), not the Sigmoid table.
 - Prologue: host-packed DMA layouts (one start for all of Wq/Wk/Wv/WgT;
   contiguous per-chunk kT/vT), projections share the pl psum pool, and the
   attention pipeline starts right after chunk-0 projections.
 - Tail: the 4 softmax denominators (rows 32/96 of the pctx banks) are
   broadcast by 4 packed K=1 matmuls into one [128,512] psum bank, one
   full-width reciprocal, grs = gate * recip, comb[128hd, 512q] = ctx * grs
   (bf16), then the output projection is 4 head-packed K=128 matmuls
   (stationary comb [128, 128q], moving WoT [128, 256]) + K=1 o_bias rank-1
   accumulate, ACT psum->sbuf copy, 2 DMA starts per 128-row block.

All heavy tensors bf16 (halved HBM traffic; bias is 8.4 MB/core); PSUM
accumulation fp32.
"""

import numpy as np

H, D, NQT, NK, C = 8, 32, 2048, 2048, 256
NQ = 512               # q rows per core (4 q-shards)
HG = 4                 # heads per core (2 head groups)
KC = NK // 128         # 16 k-chunks
NHALF = 2 * KC         # 32 half-iterations (2 head-pairs per chunk)
SCALE = D ** -0.5

_CACHE = {}
CTX_PACKED = True   # heads col-packed into 2 psum banks via tile_position
DEBUG = False       # add intermediate dumps (set before first kernel() call)

# Per-half bias path. The cost model's PE p-state ramp (full 2.4 GHz only
# after >3us of CONTINUOUS busy; any idle gap resets to 1.2/0.65 GHz) makes
# an under-committed PE catastrophic: each matmul then costs 2-3x and sits on
# the exp critical path. All-inject keeps the PE saturated slightly above the
# ACT exp rate (1138ns vs ~1060ns per half at full speed), so it stays ramped
# and the loop is PE-bound just above the ACT floor. DVE/GPSIMD stay free.
PATTERN = ["inj"] * 16


def _build_nc():
    import concourse.bass as bass
    import concourse.bacc as bacc
    import concourse.tile as tile
    import concourse.mybir as mybir

    F32 = mybir.dt.float32
    F32R = mybir.dt.float32r
    BF16 = mybir.dt.bfloat16
    AF = mybir.ActivationFunctionType

    nc = bacc.Bacc("TRN2", debug=False)

    # ---- DRAM I/O (per core: head group g, q slice s) ----
    d_qT = nc.dram_tensor("qT", [128, 1024], BF16, kind="ExternalInput")
    d_wall = nc.dram_tensor("wall", [128, 1024], BF16, kind="ExternalInput")
    d_kTc = nc.dram_tensor("kTc", [4, 128, 1024], BF16, kind="ExternalInput")
    d_vTc = nc.dram_tensor("vTc", [4, 128, 1024], BF16, kind="ExternalInput")
    d_biasT = nc.dram_tensor("biasT", [KC, 128, HG * NQ], BF16, kind="ExternalInput")
    d_woT = nc.dram_tensor("woT", [128, C], BF16, kind="ExternalInput")
    d_ones = nc.dram_tensor("ones", [128, 128], F32R, kind="ExternalInput")
    d_ngb = nc.dram_tensor("ngb", [128, 1], F32, kind="ExternalInput")
    d_id = nc.dram_tensor("ident", [128, 128], BF16, kind="ExternalInput")
    d_out = nc.dram_tensor("out", [NQ, C], F32, kind="ExternalOutput")
    if DEBUG:
        d_dbg_qp = nc.dram_tensor("dbg_qp", [128, NQ], BF16, kind="ExternalOutput")
        d_dbg_kp = nc.dram_tensor("dbg_kp", [128, 4 * 512], BF16, kind="ExternalOutput")
        d_dbg_va = nc.dram_tensor("dbg_va", [128, 528], BF16, kind="ExternalOutput")
        d_dbg_wT = nc.dram_tensor("dbg_wT", [3, 128, 1024], BF16, kind="ExternalOutput")
        d_dbg_g = nc.dram_tensor("dbg_g", [128, NQ], F32, kind="ExternalOutput")
        d_dbg_den = nc.dram_tensor("dbg_den", [128, 1024], F32, kind="ExternalOutput")
        d_dbg_rsr = nc.dram_tensor("dbg_rsr", [128, 512], F32, kind="ExternalOutput")
        d_dbg_comb = nc.dram_tensor("dbg_comb", [128, 512], BF16, kind="ExternalOutput")

    with tile.TileContext(nc) as tc:
        with tc.tile_pool(name="persist", bufs=1) as pers:

            # ---------------- persistent SBUF ----------------
            qT_sb = pers.tile([128, 1024], BF16, name="qT_sb")
            wall_sb = pers.tile([128, 1024], BF16, name="wall_sb")
            kTc_sb = [pers.tile([128, 1024], BF16, name=f"kTc{c}") for c in range(4)]
            vTc_sb = [pers.tile([128, 1024], BF16, name=f"vTc{c}") for c in range(4)]
            q_projT = pers.tile([128, NQ], BF16, name="q_projT")
            k_projT = [pers.tile([128, 512], BF16, name=f"k_projT{c}")
                       for c in range(4)]
            v_aug = [pers.tile([128, 4 * HG * 33], BF16, name=f"v_aug{c}")
                     for c in range(4)]
            e_sb = pers.tile([128, NQ], BF16, name="e_sb")       # exp(-(gate pre))
            t_sb = pers.tile([128, NQ], F32, name="t_sb")        # 1 + e
            g_sb = pers.tile([128, NQ], F32, name="g_sb")        # sigmoid gate
            woT_sb = pers.tile([128, C], BF16, name="woT_sb")
            ones_sb = pers.tile([128, 128], F32R, name="ones_sb")
            ngb_sb = pers.tile([128, 1], F32, name="ngb_sb")     # -gating_b
            id_sb = pers.tile([128, 128], BF16, name="id_sb")
            denTP = pers.tile([128, 1024], F32R, name="denTP")
            rsr = pers.tile([128, 512], F32, name="rsr")
            grs = pers.tile([128, 512], F32, name="grs")
            comb = pers.tile([128, 512], BF16, name="comb")
            out_sb = [pers.tile([128, C], F32, name=f"out_sb{i}") for i in range(4)]

            ones_r = ones_sb[:]
            ngb_ap = ngb_sb[:]

            # pools
            bias_pool = tc.alloc_tile_pool(name="bias_sb", bufs=8)
            wT_pool = tc.alloc_tile_pool(name="wT_sb", bufs=6)
            s_pool = tc.alloc_tile_pool(name="s_sb", bufs=4)
            pl_pool = tc.alloc_tile_pool(name="pl", bufs=(3 if CTX_PACKED else 2),
                                         space="PSUM")
            pctx_pool = tc.alloc_tile_pool(name="pctx", bufs=1, space="PSUM")
            pctx = [pctx_pool.tile([128, NQ if CTX_PACKED else 2 * NQ], F32,
                                   name=f"pctx{i}") for i in range(2)]

            def ctx_ap2(h, r0, r1):
                # head h of pair h//2 lives at partition 64*(h%2) of its bank
                return pctx[h // 2][64 * (h % 2) + r0:64 * (h % 2) + r1, :]

            bias_tiles = {}

            # half-major iteration order: i = 0..31 -> (half = i//16 head
            # pair, kc = i%16). Heads 0/1 finish accumulating at i=15, so
            # their den/recip/gate/comb chain runs DURING the second
            # half-loop on otherwise-idle DVE/ACT slots.
            def ihk(i):
                return i % 16, i // 16

            def emit_dma(i):
                kc, half = ihk(i)
                bt = bias_pool.tile([128, 2 * NQ], BF16, tag="bias", name="bias_t")
                nc.sync.dma_start(
                    bt[:], d_biasT.ap()[kc, :, half * 2 * NQ:(half + 1) * 2 * NQ])
                bias_tiles[i] = bt

            # ---------------- prologue DMAs ----------------
            # sync (SP) queue in submission-priority order: DMA engines drain
            # descriptors roughly FIFO, so arrival tracks submission. q-side
            # first (gates the first matmul), then k/v chunks interleaved
            # with the first bias tiles; gpsimd queue: tail-only constants.
            nc.sync.dma_start(qT_sb[:], d_qT.ap()[:])
            nc.sync.dma_start(wall_sb[:], d_wall.ap()[:])
            nc.sync.dma_start(ngb_sb[:], d_ngb.ap()[:])
            nc.gpsimd.dma_start(id_sb[:], d_id.ap()[:])
            nc.gpsimd.dma_start(ones_sb[:], d_ones.ap()[:])
            nc.gpsimd.dma_start(woT_sb[:], d_woT.ap()[:])
            nc.sync.dma_start(kTc_sb[0][:], d_kTc.ap()[0])
            nc.sync.dma_start(vTc_sb[0][:], d_vTc.ap()[0])
            emit_dma(0)
            emit_dma(1)
            nc.sync.dma_start(kTc_sb[1][:], d_kTc.ap()[1])
            nc.sync.dma_start(vTc_sb[1][:], d_vTc.ap()[1])
            emit_dma(2)
            emit_dma(3)
            nc.sync.dma_start(kTc_sb[2][:], d_kTc.ap()[2])
            nc.sync.dma_start(vTc_sb[2][:], d_vTc.ap()[2])
            emit_dma(4)
            nc.sync.dma_start(kTc_sb[3][:], d_kTc.ap()[3])
            nc.sync.dma_start(vTc_sb[3][:], d_vTc.ap()[3])
            for i in range(5, 8):
                emit_dma(i)

            # weight slice views within wall_sb: [wq0 wq1 wk0 wk1 wv0 wv1 wg0 wg1]
            def wsl(idx):
                return wall_sb[:, idx * 128:(idx + 1) * 128]

            # v_aug ones columns (position 32 of each 33-wide block)
            v_aug4w = [v_aug[c].rearrange("p (n h e) -> p n h e", n=4, h=HG)
                       for c in range(4)]
            for c in range(4):
                nc.vector.tensor_copy(
                    v_aug4w[c][:, :, :, D:D + 1],
                    ones_r[:, 0:4 * HG].bitcast(F32).rearrange(
                        "p (n h) -> p n h", n=4).unsqueeze(-1))

            # ---------------- projections ----------------
            # q_proj + gate share one pl tile; per-chunk k/v share one pl tile
            def emit_qg():
                pqg = pl_pool.tile([128, 2 * NQ], F32, tag="pl", name="pqg")
                pq, pg = pqg[:, 0:NQ], pqg[:, NQ:2 * NQ]
                for kk in range(2):
                    nc.tensor.matmul(pq, wsl(kk), qT_sb[:, kk * 512:(kk + 1) * 512],
                                     start=(kk == 0), stop=(kk == 1))
                for kk in range(2):
                    nc.tensor.matmul(pg, wsl(6 + kk), qT_sb[:, kk * 512:(kk + 1) * 512],
                                     start=(kk == 0), stop=(kk == 1))
                nc.vector.tensor_copy(q_projT[:], pq)
                # gate: e = exp(-(pre + gb)) on the Exp table; finish on DVE
                nc.scalar.activation(e_sb[:], pg, AF.Exp, bias=ngb_ap, scale=-1.0)

            def emit_proj(c):
                pkv = pl_pool.tile([128, 2 * NQ], F32, tag="pl", name="pkv")
                pk, pv = pkv[:, 0:512], pkv[:, 512:1024]
                for kk in range(2):
                    nc.tensor.matmul(pk, wsl(2 + kk),
                                     kTc_sb[c][:, kk * 512:(kk + 1) * 512],
                                     start=(kk == 0), stop=(kk == 1))
                for j in range(4):
                    for kk in range(2):
                        nc.tensor.matmul(pv[:, j * 128:(j + 1) * 128],
                                         vTc_sb[c][:, kk * 512 + j * 128:
                                                    kk * 512 + j * 128 + 128],
                                         wsl(4 + kk), start=(kk == 0), stop=(kk == 1))
                nc.vector.tensor_copy(k_projT[c][:], pk)
                nc.vector.tensor_copy(
                    v_aug4w[c][:, :, :, 0:D],
                    pv.rearrange("p (n h d) -> p n h d", n=4, h=HG))

            # ---------------- attention pipeline ----------------
            front = {}

            def emit_front(i):
                kc, half = ihk(i)
                path = PATTERN[i % 16]
                bt = bias_tiles[i]
                pl = pl_pool.tile([128, 2 * NQ], F32, tag="pl", name="pl")
                if path == "inj":
                    for j in range(2):
                        nc.tensor.matmul(pl[:, j * NQ:(j + 1) * NQ], id_sb[:],
                                         bt[:, j * NQ:(j + 1) * NQ],
                                         start=True, stop=False)
                for j in range(2):
                    h = half * 2 + j
                    po = h * 32
                    nc.tensor.matmul(pl[:, j * NQ:(j + 1) * NQ],
                                     k_projT[kc // 4][po:po + 32,
                                                     (kc % 4) * 128:(kc % 4) * 128 + 128],
                                     q_projT[po:po + 32, :],
                                     start=(path != "inj"), stop=True,
                                     tile_position=(po, 0))
                front[i] = (bt, pl)

            def emit_back(i):
                kc, half = ihk(i)
                path = PATTERN[i % 16]
                bt, pl = front.pop(i)
                wT = wT_pool.tile([128, 2 * NQ], BF16, tag="wT", name="wT")
                if path == "inj":
                    nc.scalar.activation(wT[:], pl[:], AF.Exp)
                elif path == "dve":
                    s_t = s_pool.tile([128, 2 * NQ], BF16, tag="s", name="s_t")
                    nc.vector.tensor_add(s_t[:], pl[:], bt[:])
                    nc.scalar.activation(wT[:], s_t[:], AF.Exp)
                else:  # pool: bt holds exp(bias); wT = exp(logits) * expb
                    s_t = s_pool.tile([128, 2 * NQ], BF16, tag="s", name="s_t")
                    nc.scalar.activation(s_t[:], pl[:], AF.Exp)
                    nc.gpsimd.tensor_mul(wT[:], s_t[:], bt[:])
                if DEBUG and i < 3:
                    nc.sync.dma_start(d_dbg_wT.ap()[i], wT[:])
                for j in range(2):
                    h = half * 2 + j
                    co = ((kc % 4) * HG + h) * 33
                    if CTX_PACKED:
                        nc.tensor.matmul(pctx[half][64 * j:64 * j + 33, :],
                                         v_aug[kc // 4][:, co:co + 33],
                                         wT[:, j * NQ:(j + 1) * NQ],
                                         start=(kc == 0), stop=(kc == KC - 1),
                                         tile_position=(0, 64 * j))
                    else:
                        nc.tensor.matmul(pctx[h // 2][0:33,
                                                      (h % 2) * NQ:(h % 2) * NQ + NQ],
                                         v_aug[kc // 4][:, co:co + 33],
                                         wT[:, j * NQ:(j + 1) * NQ],
                                         start=(kc == 0), stop=(kc == KC - 1))

            # q/gate + chunk-0 projections, then the pipeline starts; chunk
            # 1-3 projections are interleaved into the loop (their kTc DMAs
            # land just in time, and all-inj keeps the PE dense regardless)
            emit_qg()
            emit_proj(0)
            nc.vector.tensor_scalar_add(t_sb[:], e_sb[:], 1.0)
            nc.vector.reciprocal_approx_fast(out=g_sb[:], in_=t_sb[:])

            ctxA = pers.tile([128, 512], F32, name="ctxA")

            emit_front(0)
            emit_front(1)
            for i in range(NHALF):
                if i + 8 < NHALF:
                    emit_dma(i + 8)
                emit_back(i)
                if i == 0:
                    emit_proj(1)
                elif i == 3:
                    emit_proj(2)
                elif i == 7:
                    emit_proj(3)
                elif i == 16:
                    # heads 0/1 are final: stage dens at their OWN partitions
                    # (on DVE: ACT is the exp stream, don't perturb it)
                    nc.vector.tensor_copy(denTP[32:33, 0:512], ctx_ap2(0, 32, 33))
                    nc.vector.tensor_copy(denTP[96:97, 0:512], ctx_ap2(1, 32, 33))
                elif i == 17:
                    nc.vector.tensor_copy(ctxA[0:32, :], ctx_ap2(0, 0, 32))
                    nc.vector.tensor_copy(ctxA[32:64, :], ctx_ap2(1, 0, 32))
                elif i == 18:
                    # pctx[0] bank is now free: broadcast den h0 into rows
                    # 0:32 (K=1 rank-1 at tile row 32, baseline-proven combo)
                    nc.tensor.matmul(pctx[0][0:32, :], ones_r[32:33, 0:32],
                                     denTP[32:33, 0:512], start=True, stop=True,
                                     tile_position=(32, 0))
                elif i == 19:
                    nc.vector.tensor_copy(rsr[0:32, :], pctx[0][0:32, :])
                elif i == 20:
                    nc.tensor.matmul(pctx[0][0:32, :], ones_r[96:97, 0:32],
                                     denTP[96:97, 0:512], start=True, stop=True,
                                     tile_position=(96, 0))
                elif i == 21:
                    nc.vector.tensor_copy(rsr[32:64, :], pctx[0][0:32, :])
                elif i == 22:
                    nc.vector.reciprocal_approx_fast(out=rsr[0:64, :],
                                                     in_=rsr[0:64, :])
                    nc.vector.tensor_mul(grs[0:64, :], rsr[0:64, :],
                                         g_sb[0:64, :])
                elif i == 23:
                    nc.vector.tensor_mul(comb[0:32, :], ctxA[0:32, :],
                                         grs[0:32, :])
                    nc.vector.tensor_mul(comb[32:64, :], ctxA[32:64, :],
                                         grs[32:64, :])
                if i + 2 < NHALF:
                    emit_front(i + 2)

            # ---------------- tail (head pair 1 only) ----------------
            # heads 0/1 were fully reduced inside the loop (hooks i=16..23);
            # here: den h2/h3 -> serial rank-1 broadcasts through the freed
            # pctx[0] rows 0:32 -> rsr rows 64:128 -> one in-place recip
            # (rows 0:63 stale but unread) -> grs -> comb rows 64:128.
            nc.scalar.copy(denTP[32:33, 512:1024], ctx_ap2(2, 32, 33))
            nc.vector.tensor_copy(denTP[96:97, 512:1024], ctx_ap2(3, 32, 33))
            prsb = pl_pool.tile([128, 2 * NQ], F32, tag="pl", name="prsb")
            nc.tensor.matmul(prsb[0:32, 0:512], ones_r[32:33, 0:32],
                             denTP[32:33, 512:1024], start=True, stop=True,
                             tile_position=(32, 0))
            nc.tensor.matmul(prsb[0:32, 512:1024], ones_r[96:97, 0:32],
                             denTP[96:97, 512:1024], start=True, stop=True,
                             tile_position=(96, 0))
            nc.vector.tensor_copy(rsr[64:96, :], prsb[0:32, 0:512])
            nc.vector.tensor_copy(rsr[96:128, :], prsb[0:32, 512:1024])
            nc.vector.reciprocal_approx_fast(out=rsr[:], in_=rsr[:])
            nc.vector.tensor_mul(grs[64:128, :], rsr[64:128, :], g_sb[64:128, :])
            nc.vector.tensor_mul(comb[64:96, :], ctx_ap2(2, 0, 32),
                                 grs[64:96, :])
            nc.vector.tensor_mul(comb[96:128, :], ctx_ap2(3, 0, 32),
                                 grs[96:128, :])

            if DEBUG:
                nc.sync.dma_start(d_dbg_qp.ap()[:], q_projT[:])
                for c in range(4):
                    nc.sync.dma_start(d_dbg_kp.ap()[:, c * 512:(c + 1) * 512],
                                      k_projT[c][:])
                nc.sync.dma_start(d_dbg_va.ap()[:], v_aug[0][:])
                nc.sync.dma_start(d_dbg_g.ap()[:], g_sb[:])
                nc.sync.dma_start(d_dbg_den.ap()[:], denTP[:].bitcast(F32))
                nc.sync.dma_start(d_dbg_rsr.ap()[:], rsr[:])
                nc.sync.dma_start(d_dbg_comb.ap()[:], comb[:])

            # out[512, 256] = comb^T @ woT, head-packed K=128 (o_bias is
            # added on the host during the gather)
            for qm in range(4):
                pout = pl_pool.tile([128, 2 * NQ], F32, tag="pl", name="pout")
                nc.tensor.matmul(pout[:, 0:C],
                                 comb[:, qm * 128:qm * 128 + 128], woT_sb[:],
                                 start=True, stop=True)
                if qm % 2 == 0:
                    nc.scalar.copy(out_sb[qm][:], pout[:, 0:C])
                else:
                    nc.vector.tensor_copy(out_sb[qm][:], pout[:, 0:C])
                q0 = qm * 128
                nc.scalar.dma_start(d_out.ap()[q0:q0 + 64, :], out_sb[qm][0:64, :])
                nc.gpsimd.dma_start(d_out.ap()[q0 + 64:q0 + 128, :],
                                    out_sb[qm][64:128, :])

            pctx_pool.release()
            pl_pool.release()
            s_pool.release()
            wT_pool.release()
            bias_pool.release()

    nc.compile()
    return nc


def _prep_in_maps(inputs):
    import ml_dtypes
    BF = ml_dtypes.bfloat16
    q_data = np.asarray(inputs["q_data"], dtype=np.float32)
    k_data = np.asarray(inputs["k_data"], dtype=np.float32)
    v_data = np.asarray(inputs["v_data"], dtype=np.float32)
    pair_bias = np.asarray(inputs["pair_bias"], dtype=np.float32)
    Wq = np.asarray(inputs["Wq"], dtype=np.float32)
    Wk = np.asarray(inputs["Wk"], dtype=np.float32)
    Wv = np.asarray(inputs["Wv"], dtype=np.float32)
    Wg = np.asarray(inputs["Wg"], dtype=np.float32)
    Wo = np.asarray(inputs["Wo"], dtype=np.float32)
    gating_b = np.asarray(inputs["gating_b"], dtype=np.float32)
    o_bias = np.asarray(inputs["o_bias"], dtype=np.float32)

    wq_s = (Wq * np.float32(SCALE)).astype(BF)
    wk_b = Wk.astype(BF)
    wv_b = Wv.astype(BF)
    wgT_b = np.ascontiguousarray(Wg.T).astype(BF)
    woT = np.ascontiguousarray(Wo.T).astype(BF)          # [H*D, CO] hd-major
    # [chunk, 128, kk*512+kq] layouts for k/v (contiguous per chunk)
    kTc = np.ascontiguousarray(
        k_data.reshape(4, 512, 2, 128).transpose(0, 3, 2, 1)).reshape(
        4, 128, 1024).astype(BF)
    vTc = np.ascontiguousarray(
        v_data.reshape(4, 512, 2, 128).transpose(0, 3, 2, 1)).reshape(
        4, 128, 1024).astype(BF)
    # [k, h, q] bias, bf16, shared across cores (cores slice heads + q rows)
    pbT = np.ascontiguousarray(pair_bias.transpose(2, 0, 1)).astype(BF)
    ident = np.eye(128, dtype=np.float32).astype(BF)

    in_maps = []
    for c in range(8):
        g, s = c % 2, c // 2
        hsl = slice(g * HG, (g + 1) * HG)
        csl = slice(g * 128, (g + 1) * 128)
        qs = slice(s * NQ, (s + 1) * NQ)
        qT = q_data[qs, :].T                               # [256, 512]
        qTr = np.ascontiguousarray(
            qT.reshape(2, 128, NQ).transpose(1, 0, 2)).reshape(128, 1024)
        wall = np.concatenate(
            [wq_s[0:128, csl], wq_s[128:256, csl],
             wk_b[0:128, csl], wk_b[128:256, csl],
             wv_b[0:128, csl], wv_b[128:256, csl],
             wgT_b[0:128, csl], wgT_b[128:256, csl]], axis=1)
        biasT = np.ascontiguousarray(pbT[:, hsl, qs]).reshape(
            KC, 128, 2, 2 * NQ).astype(np.float32)
        # "pool"-path halves carry exp(bias) (consumed as a multiplicative
        # factor after exp(logits); GPSIMD cannot read PSUM)
        for i in range(NHALF):
            if PATTERN[i % 16] == "pool":
                kc, half = i // 2, i % 2
                biasT[kc, :, half] = np.exp(biasT[kc, :, half])
        biasT = biasT.reshape(KC, 128, HG * NQ).astype(BF)
        ngb = -np.ascontiguousarray(gating_b[hsl]).reshape(128, 1)
        in_maps.append(dict(
            qT=qTr.astype(BF), wall=np.ascontiguousarray(wall),
            kTc=kTc, vTc=vTc, biasT=biasT,
            woT=np.ascontiguousarray(woT[csl.start:csl.stop, :]),
            ones=np.ones((128, 128), dtype=np.float32),
            ngb=ngb, ident=ident,
        ))
    return in_maps


def _get_nc():
    if "nc" not in _CACHE:
        _CACHE["nc"] = _build_nc()
    return _CACHE["nc"]


def _run(inputs, trace=False, trace_cores=None):
    from concourse import bass_utils
    nc = _get_nc()
    in_maps = _prep_in_maps(inputs)
    kwargs = {}
    if trace:
        kwargs = dict(trace=True, trace_cores=trace_cores or [0])
    res = bass_utils.run_bass_kernel_spmd(nc, in_maps, core_ids=list(range(8)), **kwargs)
    # gather: sum the two head-group partials of each q slice
    out = np.concatenate(
        [res.results[2 * s]["out"] + res.results[2 * s + 1]["out"] for s in range(4)],
        axis=0)
    out += np.asarray(inputs["o_bias"], dtype=np.float32)[None, :]
    return out, res


def kernel(**inputs) -> np.ndarray:
    out, _ = _run(inputs)
    return out


# revision 58
# speedup vs baseline: 1.0776x; 1.0007x over previous
"""Trainium2 Bass kernel for nn_Attention_80092550136278.

Gated attention with pair bias:
  q = (q_data @ Wq) * d^-0.5 ; k = k_data @ Wk ; v = v_data @ Wv   (per head)
  w = softmax(q k^T + pair_bias) ; ctx = w @ v
  out = (ctx * sigmoid(q_data @ Wg.T + gating_b)) @ Wo.T + o_bias

Sharding: 2D - 4 q-shards x 2 head-groups over 8 cores. Core c handles head
group g = c % 2 (4 heads) and q rows s = c // 2 (512 rows). The two
head-group partials of each q slice are summed on the host during the gather
(head-parallel "all-reduce"); o_bias is zeroed on g=1 cores.

v4 structure (from the v3 trace: 25us prologue + 48us stalling main loop +
19.5us serial tail at 92.4us total; ACT-exp is the fundamental floor at
~34us/core):

 - Logits computed transposed [k, q]: pl[128k, 2h*512q] = bias + k_projT^T
   @ q_projT per 128-row k chunk (KC=16 chunks x 2 head-halves = 32 tiles).
 - Bias is fed three ways, round-robin, to balance engines below the ACT
   exp rate (~1.08us/tile): PE identity-matmul inject (12), DVE tensor_add
   (12), GPSIMD tensor_add (8).
 - ctx accumulators are column-packed with tile_position=(0, 64*j): heads
   2*half+j of one half share ONE psum bank [33+33 rows at partition 0/64],
   so pctx = 2 banks total (v3: 4), freeing pl_pool to 3 bufs (6 banks) for
   a deeper logits pipeline.
 - Single ACT table load: the gate sigmoid is computed via the Exp table
   (exp(-x) with scale=-1, then 1/(1+e) on DVE), not the Sigmoid table.
 - Prologue: host-packed DMA layouts (one start for all of Wq/Wk/Wv/WgT;
   contiguous per-chunk kT/vT), projections share the pl psum pool, and the
   attention pipeline starts right after chunk-0 projections.
 - Tail: the 4 softmax denominators (rows 32/96 of the pctx banks) are
   broadcast by 4 packed K=1 matmuls into one [128,512] psum bank, one
   full-width reciprocal, grs = gate * recip, comb[128hd, 512q] = ctx * grs
   (bf16), then the output projection is 4 head-packed K=128 matmuls
   (stationary comb [128, 128q], moving WoT [128, 256]) + K=1 o_bias rank-1
   accumulate, ACT psum->sbuf copy, 2 DMA starts per 128-row block.

All heavy tensors bf16 (halved HBM traffic; bias is 8.4 MB/core); PSUM
accumulation fp32.
"""

import numpy as np

H, D, NQT, NK, C = 8, 32, 2048, 2048, 256
NQ = 512               # q rows per core (4 q-shards)
HG = 4                 # heads per core (2 head groups)
KC = NK // 128         # 16 k-chunks
NHALF = 2 * KC         # 32 half-iterations (2 head-pairs per chunk)
SCALE = D ** -0.5

_CACHE = {}
CTX_PACKED = True   # heads col-packed into 2 psum banks via tile_position
DEBUG = False       # add intermediate dumps (set before first kernel() call)

# Per-half bias path. The cost model's PE p-state ramp (full 2.4 GHz only
# after >3us of CONTINUOUS busy; any idle gap resets to 1.2/0.65 GHz) makes
# an under-committed PE catastrophic: each matmul then costs 2-3x and sits on
# the exp critical path. All-inject keeps the PE saturated slightly above the
# ACT exp rate (1138ns vs ~1060ns per half at full speed), so it stays ramped
# and the loop is PE-bound just above the ACT floor. DVE/GPSIMD stay free.
PATTERN = ["inj"] * 16


def _build_nc():
    import concourse.bass as bass
    import concourse.bacc as bacc
    import concourse.tile as tile
    import concourse.mybir as mybir

    F32 = mybir.dt.float32
    F32R = mybir.dt.float32r
    BF16 = mybir.dt.bfloat16
    AF = mybir.ActivationFunctionType

    nc = bacc.Bacc("TRN2", debug=False)

    # ---- DRAM I/O (per core: head group g, q slice s) ----
    d_qT = nc.dram_tensor("qT", [128, 1024], BF16, kind="ExternalInput")
    d_wall = nc.dram_tensor("wall", [128, 1024], BF16, kind="ExternalInput")
    d_kTc = nc.dram_tensor("kTc", [4, 128, 1024], BF16, kind="ExternalInput")
    d_vTc = nc.dram_tensor("vTc", [4, 128, 1024], BF16, kind="ExternalInput")
    d_biasT = nc.dram_tensor("biasT", [KC, 128, HG * NQ], BF16, kind="ExternalInput")
    d_woT = nc.dram_tensor("woT", [128, C], BF16, kind="ExternalInput")
    d_ones = nc.dram_tensor("ones", [128, 128], F32R, kind="ExternalInput")
    d_ngb = nc.dram_tensor("ngb", [128, 1], F32, kind="ExternalInput")
    d_id = nc.dram_tensor("ident", [128, 128], BF16, kind="ExternalInput")
    d_out = nc.dram_tensor("out", [NQ, C], BF16, kind="ExternalOutput")
    if DEBUG:
        d_dbg_qp = nc.dram_tensor("dbg_qp", [128, NQ], BF16, kind="ExternalOutput")
        d_dbg_kp = nc.dram_tensor("dbg_kp", [128, 4 * 512], BF16, kind="ExternalOutput")
        d_dbg_va = nc.dram_tensor("dbg_va", [128, 528], BF16, kind="ExternalOutput")
        d_dbg_wT = nc.dram_tensor("dbg_wT", [3, 128, 1024], BF16, kind="ExternalOutput")
        d_dbg_g = nc.dram_tensor("dbg_g", [128, NQ], F32, kind="ExternalOutput")
        d_dbg_den = nc.dram_tensor("dbg_den", [128, 1024], F32, kind="ExternalOutput")
        d_dbg_rsr = nc.dram_tensor("dbg_rsr", [128, 512], F32, kind="ExternalOutput")
        d_dbg_comb = nc.dram_tensor("dbg_comb", [128, 512], BF16, kind="ExternalOutput")

    with tile.TileContext(nc) as tc:
        with tc.tile_pool(name="persist", bufs=1) as pers:

            # ---------------- persistent SBUF ----------------
            qT_sb = pers.tile([128, 1024], BF16, name="qT_sb")
            wall_sb = pers.tile([128, 1024], BF16, name="wall_sb")
            kTc_sb = [pers.tile([128, 1024], BF16, name=f"kTc{c}") for c in range(4)]
            vTc_sb = [pers.tile([128, 1024], BF16, name=f"vTc{c}") for c in range(4)]
            q_projT = pers.tile([128, NQ], BF16, name="q_projT")
            k_projT = [pers.tile([128, 512], BF16, name=f"k_projT{c}")
                       for c in range(4)]
            v_aug = [pers.tile([128, 4 * HG * 33], BF16, name=f"v_aug{c}")
                     for c in range(4)]
            e_sb = pers.tile([128, NQ], BF16, name="e_sb")       # exp(-(gate pre))
            t_sb = pers.tile([128, NQ], F32, name="t_sb")        # 1 + e
            g_sb = pers.tile([128, NQ], F32, name="g_sb")        # sigmoid gate
            woT_sb = pers.tile([128, C], BF16, name="woT_sb")
            ones_sb = pers.tile([128, 128], F32R, name="ones_sb")
            ngb_sb = pers.tile([128, 1], F32, name="ngb_sb")     # -gating_b
            id_sb = pers.tile([128, 128], BF16, name="id_sb")
            denTP = pers.tile([128, 1024], F32R, name="denTP")
            rsr = pers.tile([128, 512], F32, name="rsr")
            grs = pers.tile([128, 512], F32, name="grs")
            comb = pers.tile([128, 512], BF16, name="comb")
            out_sb = [pers.tile([128, C], BF16, name=f"out_sb{i}") for i in range(4)]

            ones_r = ones_sb[:]
            ngb_ap = ngb_sb[:]

            # pools
            bias_pool = tc.alloc_tile_pool(name="bias_sb", bufs=8)
            wT_pool = tc.alloc_tile_pool(name="wT_sb", bufs=6)
            s_pool = tc.alloc_tile_pool(name="s_sb", bufs=4)
            pl_pool = tc.alloc_tile_pool(name="pl", bufs=(3 if CTX_PACKED else 2),
                                         space="PSUM")
            pctx_pool = tc.alloc_tile_pool(name="pctx", bufs=1, space="PSUM")
            pctx = [pctx_pool.tile([128, NQ if CTX_PACKED else 2 * NQ], F32,
                                   name=f"pctx{i}") for i in range(2)]

            def ctx_ap2(h, r0, r1):
                # head h of pair h//2 lives at partition 64*(h%2) of its bank
                return pctx[h // 2][64 * (h % 2) + r0:64 * (h % 2) + r1, :]

            bias_tiles = {}

            # half-major iteration order: i = 0..31 -> (half = i//16 head
            # pair, kc = i%16). Heads 0/1 finish accumulating at i=15, so
            # their den/recip/gate/comb chain runs DURING the second
            # half-loop on otherwise-idle DVE/ACT slots.
            def ihk(i):
                return i % 16, i // 16

            def emit_dma(i):
                kc, half = ihk(i)
                bt = bias_pool.tile([128, 2 * NQ], BF16, tag="bias", name="bias_t")
                nc.sync.dma_start(
                    bt[:], d_biasT.ap()[kc, :, half * 2 * NQ:(half + 1) * 2 * NQ])
                bias_tiles[i] = bt

            # ---------------- prologue DMAs ----------------
            # sync (SP) queue in submission-priority order: DMA engines drain
            # descriptors roughly FIFO, so arrival tracks submission. q-side
            # first (gates the first matmul), then k/v chunks interleaved
            # with the first bias tiles; gpsimd queue: tail-only constants.
            nc.sync.dma_start(qT_sb[:], d_qT.ap()[:])
            nc.sync.dma_start(wall_sb[:], d_wall.ap()[:])
            nc.sync.dma_start(ngb_sb[:], d_ngb.ap()[:])
            nc.gpsimd.dma_start(id_sb[:], d_id.ap()[:])
            nc.gpsimd.dma_start(ones_sb[:], d_ones.ap()[:])
            nc.gpsimd.dma_start(woT_sb[:], d_woT.ap()[:])
            nc.sync.dma_start(kTc_sb[0][:], d_kTc.ap()[0])
            nc.sync.dma_start(vTc_sb[0][:], d_vTc.ap()[0])
            emit_dma(0)
            emit_dma(1)
            nc.sync.dma_start(kTc_sb[1][:], d_kTc.ap()[1])
            nc.sync.dma_start(vTc_sb[1][:], d_vTc.ap()[1])
            emit_dma(2)
            emit_dma(3)
            nc.sync.dma_start(kTc_sb[2][:], d_kTc.ap()[2])
            nc.sync.dma_start(vTc_sb[2][:], d_vTc.ap()[2])
            emit_dma(4)
            nc.sync.dma_start(kTc_sb[3][:], d_kTc.ap()[3])
            nc.sync.dma_start(vTc_sb[3][:], d_vTc.ap()[3])
            for i in range(5, 8):
                emit_dma(i)

            # weight slice views within wall_sb: [wq0 wq1 wk0 wk1 wv0 wv1 wg0 wg1]
            def wsl(idx):
                return wall_sb[:, idx * 128:(idx + 1) * 128]

            # v_aug ones columns (position 32 of each 33-wide block)
            v_aug4w = [v_aug[c].rearrange("p (n h e) -> p n h e", n=4, h=HG)
                       for c in range(4)]
            for c in range(4):
                nc.vector.tensor_copy(
                    v_aug4w[c][:, :, :, D:D + 1],
                    ones_r[:, 0:4 * HG].bitcast(F32).rearrange(
                        "p (n h) -> p n h", n=4).unsqueeze(-1))

            # ---------------- projections ----------------
            # q_proj + gate share one pl tile; per-chunk k/v share one pl tile
            def emit_qg():
                pqg = pl_pool.tile([128, 2 * NQ], F32, tag="pl", name="pqg")
                pq, pg = pqg[:, 0:NQ], pqg[:, NQ:2 * NQ]
                for kk in range(2):
                    nc.tensor.matmul(pq, wsl(kk), qT_sb[:, kk * 512:(kk + 1) * 512],
                                     start=(kk == 0), stop=(kk == 1))
                for kk in range(2):
                    nc.tensor.matmul(pg, wsl(6 + kk), qT_sb[:, kk * 512:(kk + 1) * 512],
                                     start=(kk == 0), stop=(kk == 1))
                nc.vector.tensor_copy(q_projT[:], pq)
                # gate: e = exp(-(pre + gb)) on the Exp table; finish on DVE
                nc.scalar.activation(e_sb[:], pg, AF.Exp, bias=ngb_ap, scale=-1.0)

            def emit_proj(c):
                pkv = pl_pool.tile([128, 2 * NQ], F32, tag="pl", name="pkv")
                pk, pv = pkv[:, 0:512], pkv[:, 512:1024]
                for kk in range(2):
                    nc.tensor.matmul(pk, wsl(2 + kk),
                                     kTc_sb[c][:, kk * 512:(kk + 1) * 512],
                                     start=(kk == 0), stop=(kk == 1))
                for j in range(4):
                    for kk in range(2):
                        nc.tensor.matmul(pv[:, j * 128:(j + 1) * 128],
                                         vTc_sb[c][:, kk * 512 + j * 128:
                                                    kk * 512 + j * 128 + 128],
                                         wsl(4 + kk), start=(kk == 0), stop=(kk == 1))
                nc.vector.tensor_copy(k_projT[c][:], pk)
                nc.vector.tensor_copy(
                    v_aug4w[c][:, :, :, 0:D],
                    pv.rearrange("p (n h d) -> p n h d", n=4, h=HG))

            # ---------------- attention pipeline ----------------
            front = {}

            def emit_front(i):
                kc, half = ihk(i)
                path = PATTERN[i % 16]
                bt = bias_tiles[i]
                pl = pl_pool.tile([128, 2 * NQ], F32, tag="pl", name="pl")
                if path == "inj":
                    for j in range(2):
                        nc.tensor.matmul(pl[:, j * NQ:(j + 1) * NQ], id_sb[:],
                                         bt[:, j * NQ:(j + 1) * NQ],
                                         start=True, stop=False)
                for j in range(2):
                    h = half * 2 + j
                    po = h * 32
                    nc.tensor.matmul(pl[:, j * NQ:(j + 1) * NQ],
                                     k_projT[kc // 4][po:po + 32,
                                                     (kc % 4) * 128:(kc % 4) * 128 + 128],
                                     q_projT[po:po + 32, :],
                                     start=(path != "inj"), stop=True,
                                     tile_position=(po, 0))
                front[i] = (bt, pl)

            def emit_back(i):
                kc, half = ihk(i)
                path = PATTERN[i % 16]
                bt, pl = front.pop(i)
                wT = wT_pool.tile([128, 2 * NQ], BF16, tag="wT", name="wT")
                if path == "inj":
                    nc.scalar.activation(wT[:], pl[:], AF.Exp)
                elif path == "dve":
                    s_t = s_pool.tile([128, 2 * NQ], BF16, tag="s", name="s_t")
                    nc.vector.tensor_add(s_t[:], pl[:], bt[:])
                    nc.scalar.activation(wT[:], s_t[:], AF.Exp)
                else:  # pool: bt holds exp(bias); wT = exp(logits) * expb
                    s_t = s_pool.tile([128, 2 * NQ], BF16, tag="s", name="s_t")
                    nc.scalar.activation(s_t[:], pl[:], AF.Exp)
                    nc.gpsimd.tensor_mul(wT[:], s_t[:], bt[:])
                if DEBUG and i < 3:
                    nc.sync.dma_start(d_dbg_wT.ap()[i], wT[:])
                for j in range(2):
                    h = half * 2 + j
                    co = ((kc % 4) * HG + h) * 33
                    if CTX_PACKED:
                        nc.tensor.matmul(pctx[half][64 * j:64 * j + 33, :],
                                         v_aug[kc // 4][:, co:co + 33],
                                         wT[:, j * NQ:(j + 1) * NQ],
                                         start=(kc == 0), stop=(kc == KC - 1),
                                         tile_position=(0, 64 * j))
                    else:
                        nc.tensor.matmul(pctx[h // 2][0:33,
                                                      (h % 2) * NQ:(h % 2) * NQ + NQ],
                                         v_aug[kc // 4][:, co:co + 33],
                                         wT[:, j * NQ:(j + 1) * NQ],
                                         start=(kc == 0), stop=(kc == KC - 1))

            # q/gate + chunk-0 projections, then the pipeline starts; chunk
            # 1-3 projections are interleaved into the loop (their kTc DMAs
            # land just in time, and all-inj keeps the PE dense regardless)
            emit_qg()
            emit_proj(0)
            nc.vector.tensor_scalar_add(t_sb[:], e_sb[:], 1.0)
            nc.vector.reciprocal_approx_fast(out=g_sb[:], in_=t_sb[:])

            ctxA = pers.tile([128, 512], F32, name="ctxA")

            emit_front(0)
            emit_front(1)
            for i in range(NHALF):
                if i + 8 < NHALF:
                    emit_dma(i + 8)
                emit_back(i)
                if i == 0:
                    emit_proj(1)
                elif i == 3:
                    emit_proj(2)
                elif i == 7:
                    emit_proj(3)
                elif i == 16:
                    # heads 0/1 are final: stage dens at their OWN partitions
                    # (on DVE: ACT is the exp stream, don't perturb it)
                    nc.vector.tensor_copy(denTP[32:33, 0:512], ctx_ap2(0, 32, 33))
                    nc.vector.tensor_copy(denTP[96:97, 0:512], ctx_ap2(1, 32, 33))
                elif i == 17:
                    nc.vector.tensor_copy(ctxA[0:32, :], ctx_ap2(0, 0, 32))
                    nc.vector.tensor_copy(ctxA[32:64, :], ctx_ap2(1, 0, 32))
                elif i == 18:
                    # pctx[0] bank is now free: broadcast den h0 into rows
                    # 0:32 (K=1 rank-1 at tile row 32, baseline-proven combo)
                    nc.tensor.matmul(pctx[0][0:32, :], ones_r[32:33, 0:32],
                                     denTP[32:33, 0:512], start=True, stop=True,
                                     tile_position=(32, 0))
                elif i == 19:
                    nc.vector.tensor_copy(rsr[0:32, :], pctx[0][0:32, :])
                elif i == 20:
                    nc.tensor.matmul(pctx[0][0:32, :], ones_r[96:97, 0:32],
                                     denTP[96:97, 0:512], start=True, stop=True,
                                     tile_position=(96, 0))
                elif i == 21:
                    nc.vector.tensor_copy(rsr[32:64, :], pctx[0][0:32, :])
                elif i == 22:
                    nc.vector.reciprocal_approx_fast(out=rsr[0:64, :],
                                                     in_=rsr[0:64, :])
                    nc.vector.tensor_mul(grs[0:64, :], rsr[0:64, :],
                                         g_sb[0:64, :])
                elif i == 23:
                    nc.vector.tensor_mul(comb[0:32, :], ctxA[0:32, :],
                                         grs[0:32, :])
                    nc.vector.tensor_mul(comb[32:64, :], ctxA[32:64, :],
                                         grs[32:64, :])
                if i + 2 < NHALF:
                    emit_front(i + 2)

            # ---------------- tail (head pair 1 only) ----------------
            # heads 0/1 were fully reduced inside the loop (hooks i=16..23);
            # here: den h2/h3 -> serial rank-1 broadcasts through the freed
            # pctx[0] rows 0:32 -> rsr rows 64:128 -> one in-place recip
            # (rows 0:63 stale but unread) -> grs -> comb rows 64:128.
            nc.scalar.copy(denTP[32:33, 512:1024], ctx_ap2(2, 32, 33))
            nc.vector.tensor_copy(denTP[96:97, 512:1024], ctx_ap2(3, 32, 33))
            prsb = pl_pool.tile([128, 2 * NQ], F32, tag="pl", name="prsb")
            nc.tensor.matmul(prsb[0:32, 0:512], ones_r[32:33, 0:32],
                             denTP[32:33, 512:1024], start=True, stop=True,
                             tile_position=(32, 0))
            nc.tensor.matmul(prsb[0:32, 512:1024], ones_r[96:97, 0:32],
                             denTP[96:97, 512:1024], start=True, stop=True,
                             tile_position=(96, 0))
            nc.vector.tensor_copy(rsr[64:96, :], prsb[0:32, 0:512])
            nc.vector.tensor_copy(rsr[96:128, :], prsb[0:32, 512:1024])
            nc.vector.reciprocal_approx_fast(out=rsr[:], in_=rsr[:])
            nc.vector.tensor_mul(grs[64:128, :], rsr[64:128, :], g_sb[64:128, :])
            nc.vector.tensor_mul(comb[64:96, :], ctx_ap2(2, 0, 32),
                                 grs[64:96, :])
            nc.vector.tensor_mul(comb[96:128, :], ctx_ap2(3, 0, 32),
                                 grs[96:128, :])

            if DEBUG:
                nc.sync.dma_start(d_dbg_qp.ap()[:], q_projT[:])
                for c in range(4):
                    nc.sync.dma_start(d_dbg_kp.ap()[:, c * 512:(c + 1) * 512],
                                      k_projT[c][:])
                nc.sync.dma_start(d_dbg_va.ap()[:], v_aug[0][:])
                nc.sync.dma_start(d_dbg_g.ap()[:], g_sb[:])
                nc.sync.dma_start(d_dbg_den.ap()[:], denTP[:].bitcast(F32))
                nc.sync.dma_start(d_dbg_rsr.ap()[:], rsr[:])
                nc.sync.dma_start(d_dbg_comb.ap()[:], comb[:])

            # out[512, 256] = comb^T @ woT, head-packed K=128 (o_bias is
            # added on the host during the gather)
            for qm in range(4):
                pout = pl_pool.tile([128, 2 * NQ], F32, tag="pl", name="pout")
                nc.tensor.matmul(pout[:, 0:C],
                                 comb[:, qm * 128:qm * 128 + 128], woT_sb[:],
                                 start=True, stop=True)
                if qm % 2 == 0:
                    nc.scalar.copy(out_sb[qm][:], pout[:, 0:C])
                else:
                    nc.vector.tensor_copy(out_sb[qm][:], pout[:, 0:C])
                q0 = qm * 128
                nc.scalar.dma_start(d_out.ap()[q0:q0 + 64, :], out_sb[qm][0:64, :])
                nc.gpsimd.dma_start(d_out.ap()[q0 + 64:q0 + 128, :],
                                    out_sb[qm][64:128, :])

            pctx_pool.release()
            pl_pool.release()
            s_pool.release()
            wT_pool.release()
            bias_pool.release()

    nc.compile()
    return nc


def _prep_in_maps(inputs):
    import ml_dtypes
    BF = ml_dtypes.bfloat16
    q_data = np.asarray(inputs["q_data"], dtype=np.float32)
    k_data = np.asarray(inputs["k_data"], dtype=np.float32)
    v_data = np.asarray(inputs["v_data"], dtype=np.float32)
    pair_bias = np.asarray(inputs["pair_bias"], dtype=np.float32)
    Wq = np.asarray(inputs["Wq"], dtype=np.float32)
    Wk = np.asarray(inputs["Wk"], dtype=np.float32)
    Wv = np.asarray(inputs["Wv"], dtype=np.float32)
    Wg = np.asarray(inputs["Wg"], dtype=np.float32)
    Wo = np.asarray(inputs["Wo"], dtype=np.float32)
    gating_b = np.asarray(inputs["gating_b"], dtype=np.float32)
    o_bias = np.asarray(inputs["o_bias"], dtype=np.float32)

    wq_s = (Wq * np.float32(SCALE)).astype(BF)
    wk_b = Wk.astype(BF)
    wv_b = Wv.astype(BF)
    wgT_b = np.ascontiguousarray(Wg.T).astype(BF)
    woT = np.ascontiguousarray(Wo.T).astype(BF)          # [H*D, CO] hd-major
    # [chunk, 128, kk*512+kq] layouts for k/v (contiguous per chunk)
    kTc = np.ascontiguousarray(
        k_data.reshape(4, 512, 2, 128).transpose(0, 3, 2, 1)).reshape(
        4, 128, 1024).astype(BF)
    vTc = np.ascontiguousarray(
        v_data.reshape(4, 512, 2, 128).transpose(0, 3, 2, 1)).reshape(
        4, 128, 1024).astype(BF)
    # [k, h, q] bias, bf16, shared across cores (cores slice heads + q rows)
    pbT = np.ascontiguousarray(pair_bias.transpose(2, 0, 1)).astype(BF)
    ident = np.eye(128, dtype=np.float32).astype(BF)

    in_maps = []
    for c in range(8):
        g, s = c % 2, c // 2
        hsl = slice(g * HG, (g + 1) * HG)
        csl = slice(g * 128, (g + 1) * 128)
        qs = slice(s * NQ, (s + 1) * NQ)
        qT = q_data[qs, :].T                               # [256, 512]
        qTr = np.ascontiguousarray(
            qT.reshape(2, 128, NQ).transpose(1, 0, 2)).reshape(128, 1024)
        wall = np.concatenate(
            [wq_s[0:128, csl], wq_s[128:256, csl],
             wk_b[0:128, csl], wk_b[128:256, csl],
             wv_b[0:128, csl], wv_b[128:256, csl],
             wgT_b[0:128, csl], wgT_b[128:256, csl]], axis=1)
        biasT = np.ascontiguousarray(pbT[:, hsl, qs]).reshape(
            KC, 128, 2, 2 * NQ).astype(np.float32)
        # "pool"-path halves carry exp(bias) (consumed as a multiplicative
        # factor after exp(logits); GPSIMD cannot read PSUM)
        for i in range(NHALF):
            if PATTERN[i % 16] == "pool":
                kc, half = i // 2, i % 2
                biasT[kc, :, half] = np.exp(biasT[kc, :, half])
        biasT = biasT.reshape(KC, 128, HG * NQ).astype(BF)
        ngb = -np.ascontiguousarray(gating_b[hsl]).reshape(128, 1)
        in_maps.append(dict(
            qT=qTr.astype(BF), wall=np.ascontiguousarray(wall),
            kTc=kTc, vTc=vTc, biasT=biasT,
            woT=np.ascontiguousarray(woT[csl.start:csl.stop, :]),
            ones=np.ones((128, 128), dtype=np.float32),
            ngb=ngb, ident=ident,
        ))
    return in_maps


def _get_nc():
    if "nc" not in _CACHE:
        _CACHE["nc"] = _build_nc()
    return _CACHE["nc"]


def _run(inputs, trace=False, trace_cores=None):
    from concourse import bass_utils
    nc = _get_nc()
    in_maps = _prep_in_maps(inputs)
    kwargs = {}
    if trace:
        kwargs = dict(trace=True, trace_cores=trace_cores or [0])
    res = bass_utils.run_bass_kernel_spmd(nc, in_maps, core_ids=list(range(8)), **kwargs)
    # gather: sum the two head-group partials of each q slice
    out = np.concatenate(
        [res.results[2 * s]["out"].astype(np.float32)
         + res.results[2 * s + 1]["out"].astype(np.float32) for s in range(4)],
        axis=0)
    out += np.asarray(inputs["o_bias"], dtype=np.float32)[None, :]
    return out, res


def kernel(**inputs) -> np.ndarray:
    out, _ = _run(inputs)
    return out
